# revision 9
# baseline (speedup 1.0000x reference)
"""DIN (deep interest network) Bass/Tile kernel for Trainium2, 8 NeuronCores.

Model (per batch row b):
    W_b, W_c = fc1_w[:E], fc1_w[E:]
    z[s,h]  = beh[b] @ W_b + (cand[b] @ W_c + fc1_b)        # [S,H]
    h       = relu(z)
    scores  = h @ fc2_w[:,0]                 (+fc2_b: softmax-invariant, dropped)
    attn    = softmax(scores)                               # [S]
    ui      = attn @ beh[b]                                 # [E]
    x       = [ui, cand[b]]                                 # [2E]
    logit   = relu(x @ mlp1_w + mlp1_b) @ mlp2_w + mlp2_b   # scalar

Sharding: data-parallel over batch across 8 cores (512 rows each); weights
replicated.

Per-core layout strategy:
  - beh rows stream in natural [S,E] layout (contiguous DMA), 16 rows/DMA.
  - PE transposes each row's [S,E] -> behT [E,S] (fp32-exact, via identity).
  - Main matmul zT[H,S] = W_b.T @ behT keeps W_b stationary.
  - relu+bias fused into the PSUM->SBUF evacuation (ACT bias path / DVE
    dual-op tensor_scalar), bias column c_b = (W_c.T candT + fc1_b)[:, b].
  - fc2 is col-tiled (tile_position) 4-ways so 4 batches' score matmuls run
    concurrently; scores land sparse in PSUM, are evacuated, then gathered
    dense ([Bblk,S], partition=batch) with small SBUF->SBUF DMAs.
  - softmax: exp with accum_out (fused row-sum); normalization deferred to
    the ui vectors (softmax denominator divides ui, not attn).
  - attn rows are PE-transposed once per block -> attnT columns, which drive
    per-batch ui matmuls (attnT col stationary, beh natural as moving rhs),
    col-tiled 4-ways; ui rows gathered dense, scaled by 1/sumexp.
  - MLP head runs per block of 128 batches as a handful of [128,128] matmuls
    on uiT / candT.
"""

import numpy as np

import concourse.bass as bass
import concourse.bacc as bacc
import concourse.tile as tile
from concourse import masks, mybir

F32 = mybir.dt.float32
BF16 = mybir.dt.bfloat16
AF = mybir.ActivationFunctionType
ALU = mybir.AluOpType

# Problem shapes (hardcoded per the task contract).
B_FULL, S, E = 4096, 200, 128
H, MH = 128, 256
N_CORES = 8
BS = B_FULL // N_CORES          # 512 batch rows per core
S0, S1 = 128, S - 128           # S split into partition chunks (128 + 72)

# Scores-path compute dtype (fp32 = exact; bf16 is the fast variant).
SC_DT = F32


def _ceil_div(a, b):
    return (a + b - 1) // b


def build_din(nc, bs=BS, bblk=128, gb=16, beh_bufs=None):
    """Emit the DIN program on `nc` for a per-core shard of `bs` rows."""
    assert bblk % 16 == 0 and bs % bblk == 0
    n_blocks = bs // bblk
    groups_per_blk = _ceil_div(bblk, gb)
    if beh_bufs is None:
        beh_bufs = groups_per_blk + 2

    beh_d = nc.dram_tensor("behavior_emb", [bs, S, E], F32, kind="ExternalInput").ap()
    cand_d = nc.dram_tensor("candidate_emb", [bs, E], F32, kind="ExternalInput").ap()
    fc1_w = nc.dram_tensor("fc1_w", [2 * E, H], F32, kind="ExternalInput").ap()
    fc1_b = nc.dram_tensor("fc1_b", [H], F32, kind="ExternalInput").ap()
    fc2_w = nc.dram_tensor("fc2_w", [H, 1], F32, kind="ExternalInput").ap()
    fc2_b = nc.dram_tensor("fc2_b", [1], F32, kind="ExternalInput").ap()  # unused
    mlp1_w = nc.dram_tensor("mlp1_w", [2 * E, MH], F32, kind="ExternalInput").ap()
    mlp1_b = nc.dram_tensor("mlp1_b", [MH], F32, kind="ExternalInput").ap()
    mlp2_w = nc.dram_tensor("mlp2_w", [MH, 1], F32, kind="ExternalInput").ap()
    mlp2_b = nc.dram_tensor("mlp2_b", [1], F32, kind="ExternalInput").ap()
    out_d = nc.dram_tensor("out", [bs], F32, kind="ExternalOutput").ap()

    with tile.TileContext(nc) as tc:
        with (
            tc.tile_pool(name="consts", bufs=1) as consts,
            tc.tile_pool(name="beh0", bufs=beh_bufs) as beh0_pool,
            tc.tile_pool(name="beh1", bufs=beh_bufs) as beh1_pool,
            tc.tile_pool(name="bht", bufs=3) as bht_pool,
            tc.tile_pool(name="hsb", bufs=3) as h_pool,
            tc.tile_pool(name="scsp", bufs=2) as scsp_pool,
            tc.tile_pool(name="uisp", bufs=2) as uisp_pool,
            tc.tile_pool(name="blk", bufs=2) as blk_pool,
            tc.tile_pool(name="ps_bt", bufs=2, space="PSUM") as ps_bt,
            tc.tile_pool(name="ps_z", bufs=2, space="PSUM") as ps_z,
            tc.tile_pool(name="ps_sc", bufs=1, space="PSUM") as ps_sc,
            tc.tile_pool(name="ps_ui", bufs=1, space="PSUM") as ps_ui,
            tc.tile_pool(name="ps_misc", bufs=2, space="PSUM") as ps_misc,
        ):
            # ---- constants ----
            ident = consts.tile([128, 128], F32)
            masks.make_identity(nc, ident[:])

            def load_const(name, src_ap, shape, dtype=F32):
                t = consts.tile(shape, F32, tag=name)
                nc.sync.dma_start(t[:], src_ap)
                if dtype != F32:
                    tf = t
                    t = consts.tile(shape, dtype, tag=name + "_cast")
                    nc.vector.tensor_copy(t[:], tf[:])
                return t

            wb_t = load_const("wb", fc1_w[0:E, :], [E, H], SC_DT)
            wc_t = load_const("wc", fc1_w[E:2 * E, :], [E, H])
            b1_t = load_const("b1", fc1_b.rearrange("(p o) -> p o", o=1), [H, 1])
            w2_t = load_const("w2", fc2_w[:, :], [H, 1], SC_DT)
            mw1a = load_const("mw1a", mlp1_w[0:E, :], [E, MH])
            mw1b = load_const("mw1b", mlp1_w[E:2 * E, :], [E, MH])
            mb1a = load_const("mb1a",
                              mlp1_b[0:H].rearrange("(p o) -> p o", o=1), [H, 1])
            mb1b = load_const("mb1b",
                              mlp1_b[H:MH].rearrange("(p o) -> p o", o=1), [H, 1])
            mw2a = load_const("mw2a", mlp2_w[0:H, :], [H, 1])
            mw2b = load_const("mw2b", mlp2_w[H:MH, :], [H, 1])
            mb2 = load_const("mb2", mlp2_b.rearrange("(p o) -> p o", o=1), [1, 1])

            for blk in range(n_blocks):
                b0 = blk * bblk

                # ---- per-block candidate prep ----
                cand_t = blk_pool.tile([bblk, E], F32, tag="cand")
                nc.sync.dma_start(cand_t[:], cand_d[b0:b0 + bblk, :])
                candT_ps = ps_misc.tile([128, 512], F32, tag="misc_ps")
                nc.tensor.transpose(candT_ps[0:E, 0:bblk], cand_t[:], ident[0:bblk, 0:bblk])
                candT_sb = blk_pool.tile([E, bblk], F32, tag="candT")
                nc.scalar.copy(candT_sb[:], candT_ps[0:E, 0:bblk])
                c_ps = ps_misc.tile([128, 512], F32, tag="misc_ps")
                nc.tensor.matmul(c_ps[0:H, 0:bblk], wc_t[:], candT_sb[:],
                                 start=True, stop=True)
                c_sb = blk_pool.tile([H, bblk], F32, tag="c_sb")
                nc.scalar.activation(c_sb[:], c_ps[0:H, 0:bblk], AF.Identity,
                                     bias=b1_t[:], scale=1.0)

                # dense per-block tiles
                sc_dense = blk_pool.tile([bblk, S], F32, tag="sc_dense")
                ui_dense = blk_pool.tile([bblk, E], F32, tag="ui_dense")

                beh0_tiles = []
                beh1_tiles = []

                sc_ps_g = None
                sc_rows = min(4, bblk)  # col-tile fan-out
                # ---- phase A: per-batch scores ----
                for bi in range(bblk):
                    g, i = divmod(bi, gb)
                    if i == 0:
                        ng = min(gb, bblk - g * gb)
                        bt0 = beh0_pool.tile([128, gb * E], F32, tag="beh0")
                        bt1 = beh1_pool.tile([S1, gb * E], F32, tag="beh1")
                        src = beh_d[b0 + g * gb: b0 + g * gb + ng]
                        nc.sync.dma_start(
                            bt0[:, 0:ng * E].rearrange("p (b e) -> p b e", e=E),
                            src[:, 0:S0, :].rearrange("b s e -> s b e"))
                        nc.sync.dma_start(
                            bt1[:, 0:ng * E].rearrange("p (b e) -> p b e", e=E),
                            src[:, S0:S, :].rearrange("b s e -> s b e"))
                        beh0_tiles.append(bt0)
                        beh1_tiles.append(bt1)

                    # transpose beh row -> behT [E, S]
                    bt_ps = ps_bt.tile([128, 512], F32, tag="bt_ps")
                    nc.tensor.transpose(bt_ps[:, 0:S0],
                                        beh0_tiles[g][:, i * E:(i + 1) * E], ident[:])
                    nc.tensor.transpose(bt_ps[:, S0:S],
                                        beh1_tiles[g][:, i * E:(i + 1) * E],
                                        ident[0:S1, 0:S1])
                    bht_sb = bht_pool.tile([E, S], SC_DT, tag="bht")
                    nc.vector.tensor_copy(bht_sb[:], bt_ps[:, 0:S])

                    # zT = W_b.T @ behT  (PSUM), then relu(z + c_b) -> SBUF
                    z_ps = ps_z.tile([128, 512], F32, tag="z_ps")
                    nc.tensor.matmul(z_ps[0:H, 0:S], wb_t[:], bht_sb[:],
                                     start=True, stop=True)
                    h_sb = h_pool.tile([H, S], SC_DT, tag="hsb")
                    if bi % 2 == 0:
                        nc.scalar.activation(h_sb[:], z_ps[0:H, 0:S], AF.Relu,
                                             bias=c_sb[:, bi:bi + 1], scale=1.0)
                    else:
                        nc.vector.tensor_scalar(h_sb[:], z_ps[0:H, 0:S],
                                                c_sb[:, bi:bi + 1], 0.0,
                                                ALU.add, ALU.max)

                    # fc2: scores col-tiled over 4 partition groups (M=32 with
                    # w2 broadcast so every partition is written), 8 batches
                    # per PSUM bank (2 col slots of 256)
                    j = bi % sc_rows
                    slot = (bi % 8) // sc_rows
                    if bi % 8 == 0:
                        sc_ps_g = ps_sc.tile([128, 512], F32, tag="sc_ps")
                    nc.tensor.matmul(
                        sc_ps_g[32 * j:32 * j + 32, slot * 256:slot * 256 + S],
                        w2_t[:, 0:1].broadcast_to((H, 32)), h_sb[:],
                        start=True, stop=True, tile_position=(0, 32 * j))

                    if bi % 8 == 7:
                        gbase = bi - 7
                        sc_sp = scsp_pool.tile([128, 512], F32, tag="scsp")
                        nc.vector.tensor_copy(
                            sc_sp[:, :]
                            .rearrange("p (sl x) -> p sl x", sl=2)[:, :, 0:S],
                            sc_ps_g[:, :]
                            .rearrange("p (sl x) -> p sl x", sl=2)[:, :, 0:S])
                        for sl in range(2):
                            nc.sync.dma_start(
                                sc_dense[gbase + sl * 4:gbase + sl * 4 + 4, :],
                                sc_sp[0:97:32, sl * 256:sl * 256 + S])

                # ---- phase B: softmax pieces + attnT ----
                attn_exp = blk_pool.tile([bblk, S], F32, tag="attn_exp")
                sumexp = blk_pool.tile([bblk, 1], F32, tag="sumexp")
                nc.scalar.activation(attn_exp[:], sc_dense[:], AF.Exp,
                                     accum_out=sumexp[:])
                rcp = blk_pool.tile([bblk, 1], F32, tag="rcp")
                nc.vector.reciprocal(rcp[:], sumexp[:])

                at_ps = ps_misc.tile([128, 512], F32, tag="misc_ps")
                nc.tensor.transpose(at_ps[0:S0, 0:bblk], attn_exp[:, 0:S0],
                                    ident[0:bblk, 0:bblk])
                nc.tensor.transpose(at_ps[0:S1, 256:256 + bblk], attn_exp[:, S0:S],
                                    ident[0:bblk, 0:bblk])
                attnT_sb = blk_pool.tile([128, 2 * bblk], F32, tag="attnT")
                nc.scalar.copy(attnT_sb[0:S0, 0:bblk], at_ps[0:S0, 0:bblk])
                nc.scalar.copy(attnT_sb[0:S1, bblk:2 * bblk],
                               at_ps[0:S1, 256:256 + bblk])

                # ---- phase C: per-batch ui matmuls ----
                ui_ps_g = None
                for bi in range(bblk):
                    g, i = divmod(bi, gb)
                    j = bi % sc_rows
                    slot = (bi % 16) // sc_rows
                    if bi % 16 == 0:
                        ui_ps_g = ps_ui.tile([128, 512], F32, tag="ui_ps")
                    outp = ui_ps_g[32 * j:32 * j + 32, slot * E:slot * E + E]
                    nc.tensor.matmul(outp,
                                     attnT_sb[:, bi:bi + 1].broadcast_to((S0, 32)),
                                     beh0_tiles[g][:, i * E:(i + 1) * E],
                                     start=True, stop=False,
                                     tile_position=(0, 32 * j))
                    nc.tensor.matmul(outp,
                                     attnT_sb[0:S1, bblk + bi:bblk + bi + 1]
                                     .broadcast_to((S1, 32)),
                                     beh1_tiles[g][:, i * E:(i + 1) * E],
                                     start=False, stop=True,
                                     tile_position=(0, 32 * j))
                    if bi % 16 == 15:
                        gbase = bi - 15
                        ui_sp = uisp_pool.tile([128, 512], F32, tag="uisp")
                        nc.vector.tensor_copy(ui_sp[:, :], ui_ps_g[:, :])
                        for sl in range(4):
                            nc.sync.dma_start(
                                ui_dense[gbase + sl * 4:gbase + sl * 4 + 4, :],
                                ui_sp[0:97:32, sl * E:sl * E + E])

                # ---- phase D: normalize ui, uiT, MLP head ----
                uis = blk_pool.tile([bblk, E], F32, tag="uis")
                nc.vector.tensor_scalar_mul(uis[:], ui_dense[:], rcp[:])
                uit_ps = ps_misc.tile([128, 512], F32, tag="misc_ps")
                nc.tensor.transpose(uit_ps[0:E, 0:bblk], uis[:],
                                    ident[0:bblk, 0:bblk])
                uiT_sb = blk_pool.tile([E, bblk], F32, tag="uiT")
                nc.scalar.copy(uiT_sb[:], uit_ps[0:E, 0:bblk])

                z2_ps = ps_misc.tile([128, 512], F32, tag="misc_ps")
                for m in range(2):
                    nc.tensor.matmul(z2_ps[:, m * bblk:(m + 1) * bblk],
                                     mw1a[:, m * H:(m + 1) * H], uiT_sb[:],
                                     start=True, stop=False)
                    nc.tensor.matmul(z2_ps[:, m * bblk:(m + 1) * bblk],
                                     mw1b[:, m * H:(m + 1) * H], candT_sb[:],
                                     start=False, stop=True)
                z2_sb = blk_pool.tile([128, 2 * bblk], F32, tag="z2")
                nc.scalar.activation(z2_sb[:, 0:bblk], z2_ps[:, 0:bblk], AF.Relu,
                                     bias=mb1a[:], scale=1.0)
                nc.scalar.activation(z2_sb[:, bblk:2 * bblk],
                                     z2_ps[:, bblk:2 * bblk], AF.Relu,
                                     bias=mb1b[:], scale=1.0)

                lg_ps = ps_misc.tile([128, 512], F32, tag="misc_ps")
                nc.tensor.matmul(lg_ps[0:1, 0:bblk], mw2a[:], z2_sb[:, 0:bblk],
                                 start=True, stop=False)
                nc.tensor.matmul(lg_ps[0:1, 0:bblk], mw2b[:],
                                 z2_sb[:, bblk:2 * bblk], start=False, stop=True)
                lg_sb = blk_pool.tile([1, bblk], F32, tag="lg")
                nc.scalar.activation(lg_sb[:], lg_ps[0:1, 0:bblk], AF.Identity,
                                     bias=mb2[:], scale=1.0)
                nc.sync.dma_start(
                    out_d[b0:b0 + bblk].rearrange("(o b) -> o b", o=1), lg_sb[:])
    return nc


_CACHE = {}


def _get_program():
    if "nc" not in _CACHE:
        nc = bacc.Bacc("TRN2", target_bir_lowering=False, debug=False,
                       num_devices=N_CORES)
        build_din(nc)
        nc.compile()
        _CACHE["nc"] = nc
    return _CACHE["nc"]


def kernel(**inputs):
    from concourse.bass_utils import run_bass_kernel_spmd

    nc = _get_program()
    beh = np.ascontiguousarray(np.asarray(inputs["behavior_emb"], dtype=np.float32))
    cand = np.ascontiguousarray(np.asarray(inputs["candidate_emb"], dtype=np.float32))
    weights = {
        k: np.ascontiguousarray(np.asarray(inputs[k], dtype=np.float32))
        for k in ("fc1_w", "fc1_b", "fc2_w", "fc2_b",
                  "mlp1_w", "mlp1_b", "mlp2_w", "mlp2_b")
    }
    in_maps = []
    for c in range(N_CORES):
        m = {"behavior_emb": beh[c * BS:(c + 1) * BS],
             "candidate_emb": cand[c * BS:(c + 1) * BS]}
        m.update(weights)
        in_maps.append(m)

    res = run_bass_kernel_spmd(nc, in_maps, core_ids=list(range(N_CORES)),
                               **_CACHE.get("run_kwargs", {}))
    _CACHE["last_results"] = res
    out = np.concatenate([r["out"] for r in res.results])
    return out.astype(np.float32)
